# revision 1
# baseline (speedup 1.0000x reference)
"""CASCADES adapter (moe_routing) on 8 TRN2 NeuronCores.

Reference computation (B=4, S=2048, D=4096, R=8, K=4):
    centroid[b] = 0.7*x[b,-1] + 0.3*mean_s x[b,s]
    w[b]        = softmax(cos(centroid[b], keys) / 0.05)
    Lam[b]      = sum_k w[b,k] * pool[k]                 # [R,R]
    out[b,s]    = gate * (x[b,s] @ V^T) @ Lam[b]^T @ U^T

Sharding: core i handles batch i//2, sequence half i%2 (1024 rows).
The only cross-core dependency is the full-sequence centroid; each core
contributes 0.3/2048*seqsum_local (+0.7*x_last on odd cores, passed as a
host-prepared aux input) and a pairwise AllReduce of [128,32] (16 KB)
produces the centroid on both cores of each pair.

Everything parameter-only is folded on the host: gate into U, the K
mixing matrices Mk = gate*U @ pool[k] (stacked as Mall^T [32,4096]), and
key normalization. On device the output matmul contracts over 32
partitions: lhsT = w-scaled 4x-replicated x_V^T tile [32,128], rhs =
Mall^T chunk [32,512].
"""

import numpy as np
from contextlib import ExitStack

B, S, D, R, K = 4, 2048, 4096, 8, 4
NCORES = 8
SH = S // 2            # rows per core
PT = 128               # partition tile
NT = SH // PT          # 8 sequence tiles per core
NCH = D // PT          # 32 d-chunks
KR = K * R             # 32

_CACHE = {}
LAST_RESULTS = None


def _build_program():
    from concourse import bacc, tile, mybir

    dt = mybir.dt.float32
    add = mybir.AluOpType.add
    mult = mybir.AluOpType.mult
    AF = mybir.ActivationFunctionType
    AX = mybir.AxisListType

    nc = bacc.Bacc("TRN2", target_bir_lowering=False, debug=False,
                   num_devices=NCORES)

    xs = nc.dram_tensor("xs", [SH, D], dt, kind="ExternalInput").ap()
    vt = nc.dram_tensor("vt", [PT, NCH * KR], dt, kind="ExternalInput").ap()
    mall = nc.dram_tensor("mall", [KR, D], dt, kind="ExternalInput").ap()
    kcols = nc.dram_tensor("kcols", [PT, K * NCH], dt, kind="ExternalInput").ap()
    aux = nc.dram_tensor("aux", [PT, NCH], dt, kind="ExternalInput").ap()
    ident = nc.dram_tensor("ident", [PT, PT], dt, kind="ExternalInput").ap()
    mask = nc.dram_tensor("mask", [KR, K], dt, kind="ExternalInput").ap()
    out = nc.dram_tensor("out", [SH, D], dt, kind="ExternalOutput").ap()

    f32r = mybir.dt.float32r
    NP_ = NT // 2      # s-tile pairs per core

    with tile.TileContext(nc) as tc, ExitStack() as c0:
        persist = c0.enter_context(tc.tile_pool(name="persist", bufs=1))
        dram = c0.enter_context(tc.tile_pool(name="dram", bufs=1, space="DRAM"))

        # ---- constants (gpsimd/SWDGE queue: don't block the x FIFO) ----
        kcols_sb = persist.tile([PT, K, NCH], dt, name="kcols_sb")
        nc.gpsimd.dma_start(kcols_sb[:], kcols[:])
        aux_sb = persist.tile([PT, NCH], dt, name="aux_sb")
        nc.gpsimd.dma_start(aux_sb[:], aux[:])
        ident_sb = persist.tile([PT, PT], dt, name="ident_sb")
        nc.gpsimd.dma_start(ident_sb[:], ident[:])
        mask_sb = persist.tile([KR, K], dt, name="mask_sb")
        nc.gpsimd.dma_start(mask_sb[:], mask[:])
        ones_sb = persist.tile([PT, KR], dt, name="ones_sb")
        nc.vector.memset(ones_sb[:], 1.0)

        # fp32r copies of the matmul weights (ACT copy performs the
        # fp32->fp32r rounding the walrus verifier requires)
        # staging tiles stay in the persist pool: releasing them would make
        # the xin pool's first allocation depend on these slow constant DMAs
        vt_sb = persist.tile([PT, NCH, KR], f32r, name="vt_sb")
        mall_sb = persist.tile([KR, D], f32r, name="mall_sb")
        vt_f32 = persist.tile([PT, NCH * KR], dt, name="vt_f32")
        nc.gpsimd.dma_start(vt_f32[:], vt[:])
        nc.scalar.copy(vt_sb[:], vt_f32[:].rearrange(
            "p (c r) -> p c r", r=KR))
        mall_f32 = persist.tile([KR, D], dt, name="mall_f32")
        nc.gpsimd.dma_start(mall_f32[:], mall[:])
        nc.scalar.copy(mall_sb[:], mall_f32[:])

        # ---- persistent intermediates ----
        stash_sb = persist.tile([KR, NP_, 2 * PT], dt, name="stash_sb")
        seqparts = persist.tile([PT, NCH, NT], dt, name="seqparts")

        # ================= read phase =================
        with ExitStack() as c1:
            xin = c1.enter_context(tc.tile_pool(name="xin", bufs=4))
            xtp = c1.enter_context(
                tc.tile_pool(name="xtp", bufs=6, space="PSUM"))
            xts = c1.enter_context(tc.tile_pool(name="xts", bufs=2))
            xvp = c1.enter_context(
                tc.tile_pool(name="xvp", bufs=2, space="PSUM"))

            for pr in range(NP_):
                # xt_all[p, ch, sub*128+s] = x[pair rows], fp32r, d-major
                xt_all = xts.tile([PT, NCH, 2 * PT], f32r, name="xt_all")
                for sub in range(2):
                    t = 2 * pr + sub
                    xtile = xin.tile([PT, D], dt, name="xtile")
                    nc.sync.dma_start(xtile[:], xs[t * PT:(t + 1) * PT, :])
                    for g in range(NCH // 4):
                        pt_ = xtp.tile([PT, 4, PT], dt, name="pt_")
                        for j in range(4):
                            ch = 4 * g + j
                            nc.tensor.transpose(
                                pt_[:, j, :],
                                xtile[:, ch * PT:(ch + 1) * PT],
                                ident_sb[:],
                            )
                        # rounds fp32 -> fp32r during the PSUM drain
                        nc.scalar.copy(
                            xt_all[:, 4 * g:4 * g + 4,
                                   sub * PT:(sub + 1) * PT],
                            pt_[:])
                        # per-chunk sequence sums (fp32, from PSUM)
                        nc.vector.tensor_reduce(
                            seqparts[:, 4 * g:4 * g + 4, t], pt_[:],
                            axis=AX.X, op=add)

                # x_V^T (4x-replicated rows) for both tiles of the pair:
                # out[kr, sub*128+s], contraction over d in fp32r
                xv_ps = xvp.tile([KR, 2 * PT], dt, name="xv_ps")
                for ch in range(NCH):
                    nc.tensor.matmul(
                        xv_ps[:], vt_sb[:, ch, :], xt_all[:, ch, :],
                        start=(ch == 0), stop=(ch == NCH - 1))
                nc.scalar.copy(stash_sb[:, pr, :], xv_ps[:])

        # ================= routing =================
        cc_sb = persist.tile([PT, NCH], dt, name="cc_sb")
        nc.vector.tensor_reduce(cc_sb[:], seqparts[:], axis=AX.X, op=add)
        nc.vector.tensor_scalar_mul(cc_sb[:], cc_sb[:], 0.3 / S)
        nc.vector.tensor_add(cc_sb[:], cc_sb[:], aux_sb[:])

        cin = dram.tile([PT, NCH], dt, name="cin")
        cout = dram.tile([PT, NCH], dt, name="cout")
        nc.sync.dma_start(cin[:], cc_sb[:])
        nc.gpsimd.collective_compute(
            "AllReduce",
            add,
            replica_groups=[[0, 1], [2, 3], [4, 5], [6, 7]],
            ins=[cin.opt()],
            outs=[cout.opt()],
        )
        c_sb = persist.tile([PT, NCH], dt, name="c_sb")
        nc.sync.dma_start(c_sb[:], cout[:])

        # per-partition partial dots: <c,kn_k> (k=0..3) and |c|^2
        partials = persist.tile([PT, K + 1], dt, name="partials")
        junk = persist.tile([PT, NCH], dt, name="junk")
        for k in range(K):
            nc.vector.tensor_mul(junk[:], c_sb[:], kcols_sb[:, k, :])
            nc.vector.tensor_reduce(
                partials[:, k:k + 1], junk[:], axis=AX.X, op=add)
        nc.vector.tensor_mul(junk[:], c_sb[:], c_sb[:])
        nc.vector.tensor_reduce(
            partials[:, K:K + 1], junk[:], axis=AX.X, op=add)

        with ExitStack() as cm, \
                tc.tile_pool(name="rps", bufs=1, space="PSUM") as rps:
            del cm
            r_ps = rps.tile([KR, K + 1], dt, name="r_ps")
            nc.tensor.matmul(r_ps[:], ones_sb[:], partials[:],
                             start=True, stop=True)
            rt_sb = persist.tile([KR, K + 1], dt, name="rt_sb")
            nc.scalar.copy(rt_sb[:], r_ps[:])

        cn = persist.tile([KR, 1], dt, name="cn")
        nc.scalar.sqrt(cn[:], rt_sb[:, K:K + 1])
        rcn = persist.tile([KR, 1], dt, name="rcn")
        nc.vector.reciprocal(rcn[:], cn[:])
        ex = persist.tile([KR, K], dt, name="ex")
        nc.vector.tensor_scalar(ex[:], rt_sb[:, 0:K], rcn[:], 1.0 / 0.05,
                                op0=mult, op1=mult)
        nc.scalar.activation(ex[:], ex[:], AF.Exp)
        ssum = persist.tile([KR, 1], dt, name="ssum")
        nc.vector.tensor_reduce(ssum[:], ex[:], axis=AX.X, op=add)
        rsum = persist.tile([KR, 1], dt, name="rsum")
        nc.vector.reciprocal(rsum[:], ssum[:])
        wmat = persist.tile([KR, K], dt, name="wmat")
        nc.vector.tensor_scalar_mul(wmat[:], ex[:], rsum[:])
        wcol = persist.tile([KR, 1], dt, name="wcol")
        junk2 = persist.tile([KR, K], dt, name="junk2")
        nc.vector.tensor_mul(junk2[:], wmat[:], mask_sb[:])
        nc.vector.tensor_reduce(wcol[:], junk2[:], axis=AX.X, op=add)

        # ================= write phase =================
        with ExitStack() as c2:
            otp = c2.enter_context(
                tc.tile_pool(name="otp", bufs=6, space="PSUM"))
            osb_pool = c2.enter_context(tc.tile_pool(name="osb", bufs=3))
            xvw_pool = c2.enter_context(tc.tile_pool(name="xvw", bufs=2))

            for t in range(NT):
                xvw = xvw_pool.tile([KR, PT], f32r, name="xvw")
                nc.scalar.mul(
                    xvw[:],
                    stash_sb[:, t // 2, (t % 2) * PT:(t % 2 + 1) * PT],
                    wcol[:])
                osb = osb_pool.tile([PT, D], dt, name="osb")
                for n in range(D // 512):
                    o_ps = otp.tile([PT, 512], dt, name="o_ps")
                    nc.tensor.matmul(
                        o_ps[:], xvw[:], mall_sb[:, n * 512:(n + 1) * 512],
                        start=True, stop=True)
                    dst = osb[:, n * 512:(n + 1) * 512]
                    if n % 2 == 0:
                        nc.scalar.copy(dst, o_ps[:])
                    else:
                        nc.vector.tensor_copy(dst, o_ps[:])
                half = D // 2
                nc.sync.dma_start(
                    out[t * PT:(t + 1) * PT, 0:half], osb[:, 0:half])
                nc.sync.dma_start(
                    out[t * PT:(t + 1) * PT, half:D], osb[:, half:D])

    nc.compile()
    return nc


def _get_program():
    if "nc" not in _CACHE:
        _CACHE["nc"] = _build_program()
    return _CACHE["nc"]


def _host_prep(x, U, V, pool, keys, gate_w, gate_b):
    """Parameter-only folding + per-core shard/aux construction."""
    f32 = np.float32
    # gate (parameter-only)
    gin = np.concatenate([U.mean(axis=0), V.mean(axis=1)]).astype(f32)
    z = gin @ gate_w[0].astype(f32) + gate_b[0].astype(f32)
    gate = f32(1.0) / (f32(1.0) + np.exp(-z, dtype=f32))
    Ug = (gate * U).astype(f32)

    # Mall^T [32, 4096]: rows 8k+j = (gate*U @ pool[k])[:, j]
    mall = np.concatenate(
        [(Ug @ pool[k]).T.astype(f32) for k in range(K)], axis=0)
    mall = np.ascontiguousarray(mall, dtype=f32)

    # V^T in column-chunk layout, replicated 4x along r:
    # [p, c*KR + k*R + r] = V[r, c*128+p]
    vt = np.ascontiguousarray(
        np.tile(V.T.reshape(NCH, PT, R), (1, 1, K))
        .transpose(1, 0, 2).reshape(PT, NCH * KR),
        dtype=f32)

    # normalized keys in column layout [128, K*32]: [p, k*32+c] = kn[k, c*128+p]
    knorm = np.maximum(np.linalg.norm(keys, axis=1, keepdims=True), 1e-8)
    kn = (keys / knorm).astype(f32)
    kcols = np.ascontiguousarray(
        kn.reshape(K, NCH, PT).transpose(2, 0, 1).reshape(PT, K * NCH),
        dtype=f32)

    identity = np.eye(PT, dtype=f32)
    msk = np.zeros((KR, K), dtype=f32)
    for p in range(KR):
        msk[p, p // R] = 1.0

    shared = {"vt": vt, "mall": mall, "kcols": kcols, "ident": identity,
              "mask": msk}

    in_maps = []
    for core in range(NCORES):
        b, h = divmod(core, 2)
        xsrd = np.ascontiguousarray(x[b, h * SH:(h + 1) * SH, :], dtype=f32)
        if h == 1:
            aux = np.ascontiguousarray(
                (f32(0.7) * x[b, S - 1, :]).reshape(NCH, PT).T, dtype=f32)
        else:
            aux = np.zeros((PT, NCH), dtype=f32)
        in_maps.append({"xs": xsrd, "aux": aux, **shared})
    return in_maps


def kernel(x, U_shared, V_shared, core_pool, core_keys, gate_w, gate_b):
    global LAST_RESULTS
    from concourse import bass_utils

    x = np.asarray(x, dtype=np.float32)
    U = np.asarray(U_shared, dtype=np.float32)
    V = np.asarray(V_shared, dtype=np.float32)
    pool = np.asarray(core_pool, dtype=np.float32)
    keys = np.asarray(core_keys, dtype=np.float32)
    gw = np.asarray(gate_w, dtype=np.float32)
    gb = np.asarray(gate_b, dtype=np.float32)

    nc = _get_program()
    in_maps = _host_prep(x, U, V, pool, keys, gw, gb)
    res = bass_utils.run_bass_kernel_spmd(
        nc, in_maps, core_ids=list(range(NCORES)))
    LAST_RESULTS = res

    out = np.empty((B, S, D), dtype=np.float32)
    for core in range(NCORES):
        b, h = divmod(core, 2)
        out[b, h * SH:(h + 1) * SH, :] = res.results[core]["out"]
    return out



# revision 2
# speedup vs baseline: 1.0634x; 1.0634x over previous
"""CASCADES adapter (moe_routing) on 8 TRN2 NeuronCores — v2.

Reference computation (B=4, S=2048, D=4096, R=8, K=4):
    centroid[b] = 0.7*x[b,-1] + 0.3*mean_s x[b,s]
    w[b]        = softmax(cos(centroid[b], keys) / 0.05)
    Lam[b]      = sum_k w[b,k] * pool[k]                 # [R,R]
    out[b,s]    = gate * (x[b,s] @ V^T) @ Lam[b]^T @ U^T

Sharding: core i handles batch i//2, sequence half i%2 (1024 rows).

v2 design (vs the 186 us baseline):
- x is transposed AND cast to fp16 on the host: the device reads x^T
  d-major ([128, 32*1024] per core, chunk-major free layout), which
  kills the 256 PE transposes + PSUM drains that paced the old read
  phase, and halves read traffic.
- x_V^T accumulates in a single PSUM tile [128, 256] as 4 column-group
  slabs (tile_position=(0,32q)), so the write phase can use slab g as a
  [32,128] lhsT at partition base 32g: consecutive s-tiles hit
  different PE row groups and LDWEIGHTS overlaps in-flight matmuls.
- The 16 KB centroid pair-exchange uses remote_dma to the HBM-domain
  neighbor (core^1) instead of a mesh AllReduce (~35 us -> ~3 us).
- Routing math avoids ACT table switches entirely: the one table set
  natural_log_exp_and_others is preloaded during the read phase and
  1/|c| = exp(-0.5*ln(|c|^2)); reciprocal for the softmax norm is DVE.
- The output is computed and written as fp16 (8.4 MB/core) and upcast
  to fp32 on the host.
"""

import os
import numpy as np
from contextlib import ExitStack

EXCHANGE = os.environ.get("EXCHANGE", "shm")  # "shm" | "cc"

B, S, D, R, K = 4, 2048, 4096, 8, 4
NCORES = 8
SH = S // 2            # rows per core
PT = 128               # partition tile
NCH = D // PT          # 32 d-chunks
KR = K * R             # 32
NG = 8                 # x DMA groups (4 chunks each)
QS = SH // 4           # 256: s-columns per column-group slab
RMASK = 0xF0F0         # SDMA engines with D2D reach: valid same-die too
NSEND = bin(RMASK).count("1")

_CACHE = {}
LAST_RESULTS = None


def _build_program():
    from concourse import bacc, tile, mybir

    f32 = mybir.dt.float32
    f16 = mybir.dt.float16
    bf16 = mybir.dt.bfloat16
    u32 = mybir.dt.uint32
    i32 = mybir.dt.int32
    add = mybir.AluOpType.add
    mult = mybir.AluOpType.mult
    AF = mybir.ActivationFunctionType
    AX = mybir.AxisListType

    from concourse.ap import AP

    nc = bacc.Bacc("TRN2", target_bir_lowering=False, debug=False,
                   num_devices=NCORES, monotonic_sem_count=4)

    xs = nc.dram_tensor("xs", [PT, NCH * SH], f16, kind="ExternalInput").ap()
    vt = nc.dram_tensor("vt", [PT, NCH * KR], f16, kind="ExternalInput").ap()
    mall = nc.dram_tensor("mall", [PT, D], bf16, kind="ExternalInput").ap()
    kcols = nc.dram_tensor("kcols", [PT, K * NCH], f32,
                           kind="ExternalInput").ap()
    aux = nc.dram_tensor("aux", [PT, NCH], f32, kind="ExternalInput").ap()
    mask = nc.dram_tensor("mask", [PT, K], f32, kind="ExternalInput").ap()
    peer = nc.dram_tensor("peer", [1, 4], u32, kind="ExternalInput").ap()
    out = nc.dram_tensor("out", [SH, D], f16, kind="ExternalOutput").ap()

    SLOT = PT * NCH
    FSLOT = 16
    xsh = nc.dram_tensor("xsh", [NCORES * SLOT], f32, kind="Internal",
                         addr_space="Shared").ap()
    xflag = nc.dram_tensor("xflag", [NCORES * FSLOT], u32, kind="Internal",
                           addr_space="Shared").ap()
    s_pay = nc.monotonic_semaphore(0)
    s_poll = nc.monotonic_semaphore(1)
    s_rd = nc.monotonic_semaphore(2)

    with tile.TileContext(nc) as tc, ExitStack() as c0:
        persist = c0.enter_context(tc.tile_pool(name="persist", bufs=1))

        # ---- constants (gpsimd/SWDGE queue: keep the sync FIFO for x) ----
        vt_sb = persist.tile([PT, NCH, KR], f16, name="vt_sb")
        nc.gpsimd.dma_start(vt_sb[:], vt[:].rearrange("p (c r) -> p c r",
                                                      r=KR))
        mall_sb = persist.tile([PT, D], bf16, name="mall_sb")
        nc.gpsimd.dma_start(mall_sb[:], mall[:])
        kc_sb = persist.tile([PT, K, NCH], f32, name="kc_sb")
        nc.gpsimd.dma_start(kc_sb[:], kcols[:].rearrange("p (k c) -> p k c",
                                                         k=K))
        aux_sb = persist.tile([PT, NCH], f32, name="aux_sb")
        nc.gpsimd.dma_start(aux_sb[:], aux[:])
        mask_sb = persist.tile([PT, K], f32, name="mask_sb")
        nc.gpsimd.dma_start(mask_sb[:], mask[:])
        ones_sb = persist.tile([PT, PT], f32, name="ones_sb")
        nc.vector.memset(ones_sb[:], 1.0)

        # preload the exp ACT table set so no table load sits on the
        # post-exchange critical path (Copy lives in every set and does
        # not evict it; the |c| rsqrt runs on DVE so Exp is the only
        # table-backed ACT function in the program)
        dummy = persist.tile([1, 1], f32, name="dummy")
        nc.vector.memset(dummy[:], 1.0)
        nc.scalar.activation(dummy[:], dummy[:], AF.Exp)

        # ---- persistent intermediates ----
        seqparts = persist.tile([PT, NCH], f32, name="seqparts")
        stash4 = persist.tile([PT, QS], bf16, name="stash4")
        cc_sb = persist.tile([PT, NCH], f32, name="cc_sb")
        c_in = persist.tile([PT, NCH], f32, name="c_in")
        c_full = persist.tile([PT, NCH], f32, name="c_full")
        nonce_sb = persist.tile([1, 1], u32, name="nonce_sb")
        pollw = persist.tile([1, 1], u32, name="pollw")
        nc.gpsimd.dma_start(nonce_sb[:], peer[0:1, 2:3])
        nc.vector.memset(pollw[:], 0)

        # ================= read phase =================
        with ExitStack() as c1:
            xin = c1.enter_context(tc.tile_pool(name="xin", bufs=4))
            xvp = c1.enter_context(
                tc.tile_pool(name="xvp", bufs=1, space="PSUM"))
            ps_xv = xvp.tile([PT, QS], f32, name="ps_xv")
            junk = persist.tile([PT, SH // 4 * 4], f16, name="junk")

            for g in range(NG):
                xt = xin.tile([PT, 4, SH], f16, name="xt")
                nc.sync.dma_start(
                    xt[:],
                    xs[:, g * 4 * SH:(g + 1) * 4 * SH]
                    .rearrange("p (j s) -> p j s", j=4))
                for j in range(4):
                    ch = 4 * g + j
                    for q in range(4):
                        nc.tensor.matmul(
                            ps_xv[32 * q:32 * (q + 1), :],
                            vt_sb[:, ch, :],
                            xt[:, j, QS * q:QS * (q + 1)],
                            start=(ch == 0), stop=(ch == NCH - 1),
                            tile_position=(0, 32 * q))
                    # per-chunk sequence sums: [128, 1024] f16 -> f32
                    if ch % 2 == 0:
                        nc.vector.tensor_reduce(
                            seqparts[:, ch:ch + 1], xt[:, j, :],
                            axis=AX.X, op=add)
                    else:
                        nc.scalar.activation(
                            junk[:, 0:SH], xt[:, j, :], AF.Copy,
                            accum_out=seqparts[:, ch:ch + 1])

            # x_V^T slabs -> fp16 stash (PSUM -> SBUF)
            nc.scalar.copy(stash4[:], ps_xv[:])

        # centroid partial: 0.3/S * seqsum (+0.7*x_last via host aux)
        nc.vector.tensor_scalar_mul(cc_sb[:], seqparts[:], 0.3 / S)
        nc.vector.tensor_add(cc_sb[:], cc_sb[:], aux_sb[:])

        # ================= pair exchange =================
        if EXCHANGE == "shm":
            # write own centroid partial + nonce flag into the Shared
            # scratchpad, poll the HBM-domain neighbor's flag, read its
            # partial back. ~4 local DMA latencies instead of a ~26 us
            # mesh AllReduce.
            with tc.tile_critical():
                g = nc.gpsimd
                own_rg = g.alloc_register("own_rg")
                peer_rg = g.alloc_register("peer_rg")
                nonce_rg = g.alloc_register("nonce_rg")
                g.reg_load(own_rg, peer[0:1, 0:1])
                g.reg_load(peer_rg, peer[0:1, 1:2])
                g.reg_load(nonce_rg, peer[0:1, 2:3])
                own_sv = g.snap(own_rg, min_val=0, max_val=NCORES - 1)
                peer_sv = g.snap(peer_rg, min_val=0, max_val=NCORES - 1)

                my_pay = AP(xsh.tensor, own_sv * SLOT, [[NCH, PT], [1, NCH]])
                g.dma_start(my_pay, cc_sb[:]).then_inc(s_pay.sem(), 16)
                s_pay.wait_inc(16)

                my_flag = AP(xflag.tensor, own_sv * FSLOT, [[1, 1], [1, 1]])
                g.dma_start(my_flag, nonce_sb[:]).then_inc(s_pay.sem(), 16)
                s_pay.inc_expected(16)

                peer_flag = AP(xflag.tensor, peer_sv * FSLOT,
                               [[1, 1], [1, 1]])
                ne_rg = g.alloc_register("ne_rg")
                fl_rg = g.alloc_register("fl_rg")

                def cond():
                    g.dma_start(pollw[:], peer_flag).then_inc(
                        s_poll.sem(), 16)
                    s_poll.wait_inc(16)
                    g.reg_load(fl_rg, pollw[:])
                    g.reg_alu(ne_rg, fl_rg, nonce_rg,
                              mybir.AluOpType.not_equal)
                    return ne_rg

                with g.While(cond):
                    pass

                peer_pay = AP(xsh.tensor, peer_sv * SLOT,
                              [[NCH, PT], [1, NCH]])
                g.dma_start(c_in[:], peer_pay).then_inc(s_rd.sem(), 16)
                s_rd.wait_inc(16)
            nc.vector.tensor_add(c_full[:], cc_sb[:], c_in[:])
        else:
            with tc.tile_pool(name="dram", bufs=1, space="DRAM") as dram:
                cin = dram.tile([PT, NCH], f32, name="cin")
                cout = dram.tile([PT, NCH], f32, name="cout")
                nc.sync.dma_start(cin[:], cc_sb[:])
                nc.gpsimd.collective_compute(
                    "AllReduce",
                    add,
                    replica_groups=[[0, 1], [2, 3], [4, 5], [6, 7]],
                    ins=[cin.opt()],
                    outs=[cout.opt()],
                )
                nc.sync.dma_start(c_full[:], cout[:])

        # ================= routing =================
        junk4 = persist.tile([PT, K, NCH], f32, name="junk4")
        junkc = persist.tile([PT, NCH], f32, name="junkc")
        partials = persist.tile([PT, K + 1], f32, name="partials")
        for k in range(K):
            nc.vector.tensor_mul(junk4[:, k, :], c_full[:], kc_sb[:, k, :])
        nc.vector.tensor_reduce(partials[:, 0:K], junk4[:], axis=AX.X, op=add)
        nc.vector.tensor_mul(junkc[:], c_full[:], c_full[:])
        nc.vector.tensor_reduce(partials[:, K:K + 1], junkc[:],
                                axis=AX.X, op=add)

        rt_sb = persist.tile([PT, K + 1], f32, name="rt_sb")
        with tc.tile_pool(name="rps", bufs=1, space="PSUM") as rps:
            r_ps = rps.tile([PT, K + 1], f32, name="r_ps")
            nc.tensor.matmul(r_ps[:], ones_sb[:], partials[:],
                             start=True, stop=True)
            nc.scalar.copy(rt_sb[:], r_ps[:])

        # 1/|c| = rsqrt(|c|^2) on DVE only (no ACT table): quake seed
        # y0 = bits(0x5f3759df - (bits(ss) >> 1)) + two Newton steps
        # y <- y*(1.5 - 0.5*ss*y^2). Seed err ~3.4% -> ~4e-6 after 2 steps.
        shr = mybir.AluOpType.arith_shift_right
        bxor = mybir.AluOpType.bitwise_xor
        ssv = rt_sb[:, K:K + 1]
        rns = persist.tile([PT, 1], f32, name="rns")
        rns_i = rns[:].bitcast(i32)
        nc.vector.tensor_scalar(rns_i, ssv.bitcast(i32), 1, None, op0=shr)
        # 0x5f3759df - t == (t ^ 0xFFFFFFFF) + 0x5f3759e0
        nc.vector.tensor_scalar(rns_i, rns_i, -1, None, op0=bxor)
        nc.vector.tensor_scalar(rns_i, rns_i, 0x5f3759e0, None, op0=add)
        nwt = persist.tile([PT, 2], f32, name="nwt")
        for _ in range(2):
            nc.vector.tensor_mul(nwt[:, 0:1], rns[:], rns[:])
            nc.vector.tensor_mul(nwt[:, 1:2], nwt[:, 0:1], ssv)
            nc.vector.tensor_scalar(nwt[:, 1:2], nwt[:, 1:2], -0.5, 1.5,
                                    op0=mult, op1=add)
            nc.vector.tensor_mul(rns[:], rns[:], nwt[:, 1:2])
        ex = persist.tile([PT, K], f32, name="ex")
        nc.vector.tensor_scalar(ex[:], rt_sb[:, 0:K], rns[:], 1.0 / 0.05,
                                op0=mult, op1=mult)
        nc.scalar.activation(ex[:], ex[:], AF.Exp)
        ssum = persist.tile([PT, 1], f32, name="ssum")
        nc.vector.tensor_reduce(ssum[:], ex[:], axis=AX.X, op=add)
        rsum = persist.tile([PT, 1], f32, name="rsum")
        nc.vector.reciprocal(rsum[:], ssum[:])
        wmat = persist.tile([PT, K], f32, name="wmat")
        nc.vector.tensor_scalar_mul(wmat[:], ex[:], rsum[:])
        wj = persist.tile([PT, K], f32, name="wj")
        nc.vector.tensor_mul(wj[:], wmat[:], mask_sb[:])
        wcol4 = persist.tile([PT, 1], f32, name="wcol4")
        nc.vector.tensor_reduce(wcol4[:], wj[:], axis=AX.X, op=add)

        # w-scaled x_V^T slabs, bf16 for the output matmul
        xvw4 = persist.tile([PT, QS], bf16, name="xvw4")
        nc.scalar.mul(xvw4[:], stash4[:], wcol4[:])

        # ================= write phase =================
        with ExitStack() as c2:
            otp = c2.enter_context(
                tc.tile_pool(name="otp", bufs=6, space="PSUM"))
            osb_pool = c2.enter_context(tc.tile_pool(name="osb", bufs=3))

            for t in (0, 2, 4, 6, 1, 3, 5, 7):
                g, half = t // 2, t % 2
                lhsT = xvw4[32 * g:32 * (g + 1),
                            half * PT:(half + 1) * PT]
                osb = osb_pool.tile([PT, D], f16, name="osb")
                for n in range(D // 512):
                    o_ps = otp.tile([PT, 512], f32, name="o_ps")
                    nc.tensor.matmul(
                        o_ps[:], lhsT,
                        mall_sb[32 * g:32 * (g + 1),
                                n * 512:(n + 1) * 512],
                        start=True, stop=True,
                        tile_position=(32 * g, 0))
                    dst = osb[:, n * 512:(n + 1) * 512]
                    if n % 2 == 0:
                        nc.scalar.copy(dst, o_ps[:])
                    else:
                        nc.vector.tensor_copy(dst, o_ps[:])
                half_d = D // 2
                nc.sync.dma_start(
                    out[t * PT:(t + 1) * PT, 0:half_d], osb[:, 0:half_d])
                nc.sync.dma_start(
                    out[t * PT:(t + 1) * PT, half_d:D], osb[:, half_d:D])

    nc.compile()
    return nc


def _get_program():
    if "nc" not in _CACHE:
        _CACHE["nc"] = _build_program()
    return _CACHE["nc"]


def _host_prep(x, U, V, pool, keys, gate_w, gate_b):
    """Parameter folding + per-core shard/layout construction."""
    f32 = np.float32
    f16 = np.float16
    # gate (parameter-only)
    gin = np.concatenate([U.mean(axis=0), V.mean(axis=1)]).astype(f32)
    z = gin @ gate_w[0].astype(f32) + gate_b[0].astype(f32)
    gate = f32(1.0) / (f32(1.0) + np.exp(-z, dtype=f32))
    Ug = (gate * U).astype(f32)

    # mall4 [128, 4096] bf16: 4 replicated slabs of Mall^T [32, 4096],
    # rows 8k+j = (gate*U @ pool[k])[:, j]
    import ml_dtypes
    mall = np.concatenate(
        [(Ug @ pool[k]).T.astype(f32) for k in range(K)], axis=0)
    mall4 = np.ascontiguousarray(np.tile(mall, (4, 1))).astype(
        ml_dtypes.bfloat16)

    # V^T chunk-major, replicated 4x along r: vt[p, c*KR + k*R + r]
    # = V[r, c*128+p]
    vt = np.ascontiguousarray(
        np.tile(V.T.reshape(NCH, PT, R), (1, 1, K))
        .transpose(1, 0, 2).reshape(PT, NCH * KR)).astype(f16)

    # normalized keys, chunk layout [128, K*32]: [p, k*32+c] = kn[k, c*128+p]
    knorm = np.maximum(np.linalg.norm(keys, axis=1, keepdims=True), 1e-8)
    kn = (keys / knorm).astype(f32)
    kcols = np.ascontiguousarray(
        kn.reshape(K, NCH, PT).transpose(2, 0, 1).reshape(PT, K * NCH),
        dtype=f32)

    # mask4 [128, 4]: partition p contributes to expert (p%32)//8
    msk = np.zeros((PT, K), dtype=f32)
    for p in range(PT):
        msk[p, (p % KR) // R] = 1.0

    shared = {"vt": vt, "mall": mall4, "kcols": kcols, "mask": msk}

    # fresh per-call nonce: stale Shared-scratchpad flags from a previous
    # call must never match this call's handshake
    nonce = np.uint32(int.from_bytes(os.urandom(4), "little") | 1)

    in_maps = []
    for core in range(NCORES):
        b, h = divmod(core, 2)
        # x^T fp16, chunk-major: xs[p, c*1024+s] = x[b, h*1024+s, c*128+p]
        xsrd = np.ascontiguousarray(
            x[b, h * SH:(h + 1) * SH, :].T.reshape(NCH, PT, SH)
            .transpose(1, 0, 2).reshape(PT, NCH * SH)).astype(f16)
        if h == 1:
            auxc = np.ascontiguousarray(
                (f32(0.7) * x[b, S - 1, :]).reshape(NCH, PT).T, dtype=f32)
        else:
            auxc = np.zeros((PT, NCH), dtype=f32)
        pr = np.zeros((1, 4), dtype=np.uint32)
        pr[0, 0] = core
        pr[0, 1] = core ^ 1
        pr[0, 2] = nonce
        in_maps.append({"xs": xsrd, "aux": auxc, "peer": pr, **shared})
    return in_maps


def kernel(x, U_shared, V_shared, core_pool, core_keys, gate_w, gate_b):
    global LAST_RESULTS
    from concourse import bass_utils

    x = np.asarray(x, dtype=np.float32)
    U = np.asarray(U_shared, dtype=np.float32)
    V = np.asarray(V_shared, dtype=np.float32)
    pool = np.asarray(core_pool, dtype=np.float32)
    keys = np.asarray(core_keys, dtype=np.float32)
    gw = np.asarray(gate_w, dtype=np.float32)
    gb = np.asarray(gate_b, dtype=np.float32)

    nc = _get_program()
    in_maps = _host_prep(x, U, V, pool, keys, gw, gb)
    res = bass_utils.run_bass_kernel_spmd(
        nc, in_maps, core_ids=list(range(NCORES)))
    LAST_RESULTS = res

    out = np.empty((B, S, D), dtype=np.float32)
    for core in range(NCORES):
        b, h = divmod(core, 2)
        out[b, h * SH:(h + 1) * SH, :] = res.results[core]["out"]
    return out


# revision 3
# speedup vs baseline: 1.0728x; 1.0088x over previous
"""CASCADES adapter (moe_routing) on 8 TRN2 NeuronCores — v2.

Reference computation (B=4, S=2048, D=4096, R=8, K=4):
    centroid[b] = 0.7*x[b,-1] + 0.3*mean_s x[b,s]
    w[b]        = softmax(cos(centroid[b], keys) / 0.05)
    Lam[b]      = sum_k w[b,k] * pool[k]                 # [R,R]
    out[b,s]    = gate * (x[b,s] @ V^T) @ Lam[b]^T @ U^T

Sharding: core i handles batch i//2, sequence half i%2 (1024 rows).

v2 design (vs the 186 us baseline):
- x is transposed AND cast to fp16 on the host: the device reads x^T
  d-major ([128, 32*1024] per core, chunk-major free layout), which
  kills the 256 PE transposes + PSUM drains that paced the old read
  phase, and halves read traffic.
- x_V^T accumulates in a single PSUM tile [128, 256] as 4 column-group
  slabs (tile_position=(0,32q)), so the write phase can use slab g as a
  [32,128] lhsT at partition base 32g: consecutive s-tiles hit
  different PE row groups and LDWEIGHTS overlaps in-flight matmuls.
- The 16 KB centroid pair-exchange uses remote_dma to the HBM-domain
  neighbor (core^1) instead of a mesh AllReduce (~35 us -> ~3 us).
- Routing math avoids ACT table switches entirely: the one table set
  natural_log_exp_and_others is preloaded during the read phase and
  1/|c| = exp(-0.5*ln(|c|^2)); reciprocal for the softmax norm is DVE.
- The output is computed and written as fp16 (8.4 MB/core) and upcast
  to fp32 on the host.
"""

import os
import numpy as np
from contextlib import ExitStack

EXCHANGE = os.environ.get("EXCHANGE", "shm")  # "shm" | "cc"

B, S, D, R, K = 4, 2048, 4096, 8, 4
NCORES = 8
SH = S // 2            # rows per core
PT = 128               # partition tile
NCH = D // PT          # 32 d-chunks
KR = K * R             # 32
NG = 8                 # x DMA groups (4 chunks each)
QS = SH // 4           # 256: s-columns per column-group slab
RMASK = 0xF0F0         # SDMA engines with D2D reach: valid same-die too
NSEND = bin(RMASK).count("1")

_CACHE = {}
LAST_RESULTS = None


def _build_program():
    from concourse import bacc, tile, mybir

    f32 = mybir.dt.float32
    f16 = mybir.dt.float16
    bf16 = mybir.dt.bfloat16
    u32 = mybir.dt.uint32
    i32 = mybir.dt.int32
    add = mybir.AluOpType.add
    mult = mybir.AluOpType.mult
    AF = mybir.ActivationFunctionType
    AX = mybir.AxisListType

    from concourse.ap import AP

    nc = bacc.Bacc("TRN2", target_bir_lowering=False, debug=False,
                   num_devices=NCORES, monotonic_sem_count=4,
                   enable_partition_id=False)

    xs = nc.dram_tensor("xs", [PT, NCH * SH], f16, kind="ExternalInput").ap()
    vt = nc.dram_tensor("vt", [PT, NCH * KR], f16, kind="ExternalInput").ap()
    mall = nc.dram_tensor("mall", [PT, D], bf16, kind="ExternalInput").ap()
    kcols = nc.dram_tensor("kcols", [PT, K * NCH], f32,
                           kind="ExternalInput").ap()
    aux = nc.dram_tensor("aux", [PT, NCH], f32, kind="ExternalInput").ap()
    mask = nc.dram_tensor("mask", [PT, K], f32, kind="ExternalInput").ap()
    peer = nc.dram_tensor("peer", [1, 4], u32, kind="ExternalInput").ap()
    out = nc.dram_tensor("out", [SH, D], f16, kind="ExternalOutput").ap()

    SLOT = PT * NCH
    FSLOT = 16
    xsh = nc.dram_tensor("xsh", [NCORES * SLOT], f32, kind="Internal",
                         addr_space="Shared").ap()
    xflag = nc.dram_tensor("xflag", [NCORES * FSLOT], u32, kind="Internal",
                           addr_space="Shared").ap()
    s_pay = nc.monotonic_semaphore(0)
    s_poll = nc.monotonic_semaphore(1)
    s_rd = nc.monotonic_semaphore(2)

    with tile.TileContext(nc) as tc, ExitStack() as c0:
        persist = c0.enter_context(tc.tile_pool(name="persist", bufs=1))

        # ---- constants (gpsimd/SWDGE queue: keep the sync FIFO for x) ----
        vt_sb = persist.tile([PT, NCH, KR], f16, name="vt_sb")
        nc.gpsimd.dma_start(vt_sb[:], vt[:].rearrange("p (c r) -> p c r",
                                                      r=KR))
        mall_sb = persist.tile([PT, D], bf16, name="mall_sb")
        nc.gpsimd.dma_start(mall_sb[:], mall[:])
        kc_sb = persist.tile([PT, K, NCH], f32, name="kc_sb")
        nc.gpsimd.dma_start(kc_sb[:], kcols[:].rearrange("p (k c) -> p k c",
                                                         k=K))
        aux_sb = persist.tile([PT, NCH], f32, name="aux_sb")
        nc.gpsimd.dma_start(aux_sb[:], aux[:])
        mask_sb = persist.tile([PT, K], f32, name="mask_sb")
        nc.gpsimd.dma_start(mask_sb[:], mask[:])
        ones_sb = persist.tile([PT, PT], f32, name="ones_sb")
        nc.vector.memset(ones_sb[:], 1.0)

        # preload the exp ACT table set so no table load sits on the
        # post-exchange critical path (Copy lives in every set and does
        # not evict it; the |c| rsqrt runs on DVE so Exp is the only
        # table-backed ACT function in the program)
        dummy = persist.tile([1, 1], f32, name="dummy")
        nc.vector.memset(dummy[:], 1.0)
        nc.scalar.activation(dummy[:], dummy[:], AF.Exp)

        # ---- persistent intermediates ----
        seqparts = persist.tile([PT, NCH], f32, name="seqparts")
        stash4 = persist.tile([PT, QS], bf16, name="stash4")
        cc_sb = persist.tile([PT, NCH], f32, name="cc_sb")
        c_in = persist.tile([PT, NCH], f32, name="c_in")
        c_full = persist.tile([PT, NCH], f32, name="c_full")
        ids_sb = persist.tile([1, 4], u32, name="ids_sb")
        nc.gpsimd.dma_start(ids_sb[:], peer[:])
        # exchange routing registers: loaded during the read phase so the
        # post-read critical section starts straight at the payload DMA
        g = nc.gpsimd
        own_rg = g.alloc_register("own_rg")
        peer_rg = g.alloc_register("peer_rg")
        nonce_rg = g.alloc_register("nonce_rg")
        g.reg_load(own_rg, ids_sb[0:1, 0:1])
        g.reg_load(peer_rg, ids_sb[0:1, 1:2])
        g.reg_load(nonce_rg, ids_sb[0:1, 2:3])
        own_sv = g.snap(own_rg, min_val=0, max_val=NCORES - 1)
        peer_sv = g.snap(peer_rg, min_val=0, max_val=NCORES - 1)

        # ================= read phase =================
        with ExitStack() as c1:
            xin = c1.enter_context(tc.tile_pool(name="xin", bufs=6))
            xvp = c1.enter_context(
                tc.tile_pool(name="xvp", bufs=1, space="PSUM"))
            ps_xv = xvp.tile([PT, QS], f32, name="ps_xv")
            junk = persist.tile([PT, SH // 4 * 4], f16, name="junk")

            for gi in range(2 * NG):
                xt = xin.tile([PT, 2, SH], f16, name="xt")
                nc.sync.dma_start(
                    xt[:],
                    xs[:, gi * 2 * SH:(gi + 1) * 2 * SH]
                    .rearrange("p (j s) -> p j s", j=2))
                for j in range(2):
                    ch = 2 * gi + j
                    for q in range(4):
                        nc.tensor.matmul(
                            ps_xv[32 * q:32 * (q + 1), :],
                            vt_sb[:, ch, :],
                            xt[:, j, QS * q:QS * (q + 1)],
                            start=(ch == 0), stop=(ch == NCH - 1),
                            tile_position=(0, 32 * q))
                # per-half-group sequence sums [128, 1024] f16 -> f32,
                # one chunk on DVE + one on ACT so neither paces the
                # stream and the post-stream backlog stays ~1 chunk deep
                ch0 = 2 * gi
                nc.vector.tensor_reduce(
                    seqparts[:, ch0:ch0 + 1], xt[:, 0, :],
                    axis=AX.X, op=add)
                nc.scalar.activation(
                    junk[:, 0:SH], xt[:, 1, :], AF.Copy,
                    accum_out=seqparts[:, ch0 + 1:ch0 + 2])

            # x_V^T slabs -> fp16 stash (PSUM -> SBUF)
            nc.scalar.copy(stash4[:], ps_xv[:])

        # centroid partial: 0.3/S * seqsum (+0.7*x_last via host aux)
        nc.vector.tensor_scalar_mul(cc_sb[:], seqparts[:], 0.3 / S)
        nc.vector.tensor_add(cc_sb[:], cc_sb[:], aux_sb[:])

        # ================= pair exchange =================
        if EXCHANGE == "shm":
            # write own centroid partial + nonce flag into the Shared
            # scratchpad, poll the HBM-domain neighbor's flag, read its
            # partial back. ~4 local DMA latencies instead of a ~26 us
            # mesh AllReduce.
            with tc.tile_critical(no_gpsimd_drain=True):
                my_pay = AP(xsh.tensor, own_sv * SLOT, [[NCH, PT], [1, NCH]])
                g.dma_start(my_pay, cc_sb[:]).then_inc(s_pay.sem(), 16)
                s_pay.wait_inc(16)

                my_flag = AP(xflag.tensor, own_sv * FSLOT, [[1, 1], [1, 1]])
                g.dma_start(my_flag, ids_sb[0:1, 2:3]).then_inc(
                    s_pay.sem(), 16)
                s_pay.inc_expected(16)

                peer_flag = AP(xflag.tensor, peer_sv * FSLOT,
                               [[1, 1], [1, 1]])
                ne_rg = g.alloc_register("ne_rg")
                fl_rg = g.alloc_register("fl_rg")

                def cond():
                    g.reg_load(fl_rg, peer_flag)
                    g.reg_alu(ne_rg, fl_rg, nonce_rg,
                              mybir.AluOpType.not_equal)
                    return ne_rg

                with g.While(cond):
                    pass

                peer_pay = AP(xsh.tensor, peer_sv * SLOT,
                              [[NCH, PT], [1, NCH]])
                g.dma_start(c_in[:], peer_pay).then_inc(s_rd.sem(), 16)
                s_rd.wait_inc(16)
            nc.vector.tensor_add(c_full[:], cc_sb[:], c_in[:])
        else:
            with tc.tile_pool(name="dram", bufs=1, space="DRAM") as dram:
                cin = dram.tile([PT, NCH], f32, name="cin")
                cout = dram.tile([PT, NCH], f32, name="cout")
                nc.sync.dma_start(cin[:], cc_sb[:])
                nc.gpsimd.collective_compute(
                    "AllReduce",
                    add,
                    replica_groups=[[0, 1], [2, 3], [4, 5], [6, 7]],
                    ins=[cin.opt()],
                    outs=[cout.opt()],
                )
                nc.sync.dma_start(c_full[:], cout[:])

        # ================= routing =================
        junk4 = persist.tile([PT, K, NCH], f32, name="junk4")
        junkc = persist.tile([PT, NCH], f32, name="junkc")
        partials = persist.tile([PT, K + 1], f32, name="partials")
        for k in range(K):
            nc.vector.tensor_mul(junk4[:, k, :], c_full[:], kc_sb[:, k, :])
        nc.vector.tensor_reduce(partials[:, 0:K], junk4[:], axis=AX.X, op=add)
        nc.vector.tensor_mul(junkc[:], c_full[:], c_full[:])
        nc.vector.tensor_reduce(partials[:, K:K + 1], junkc[:],
                                axis=AX.X, op=add)

        rps = c0.enter_context(tc.tile_pool(name="rps", bufs=1, space="PSUM"))
        r_ps = rps.tile([PT, K + 1], f32, name="r_ps")
        nc.tensor.matmul(r_ps[:], ones_sb[:], partials[:],
                         start=True, stop=True)

        # 1/|c| = rsqrt(|c|^2) on DVE only (no ACT table): quake seed
        # y0 = bits(0x5f3759df - (bits(ss) >> 1)) + two Newton steps
        # y <- y*(1.5 - 0.5*ss*y^2). Seed err ~3.4% -> ~4e-6 after 2 steps.
        # DVE reads the partition-reduced dots straight from PSUM.
        shr = mybir.AluOpType.arith_shift_right
        bxor = mybir.AluOpType.bitwise_xor
        ssv = r_ps[:, K:K + 1]
        rns = persist.tile([PT, 1], f32, name="rns")
        halfss = persist.tile([PT, 1], f32, name="halfss")
        nc.vector.tensor_scalar_mul(halfss[:], ssv, -0.5)
        rns_i = rns[:].bitcast(i32)
        nc.vector.tensor_scalar(rns_i, ssv.bitcast(i32), 1, None, op0=shr)
        # 0x5f3759df - t == (t ^ 0xFFFFFFFF) + 0x5f3759e0
        nc.vector.tensor_scalar(rns_i, rns_i, -1, None, op0=bxor)
        nc.vector.tensor_scalar(rns_i, rns_i, 0x5f3759e0, None, op0=add)
        nwt = persist.tile([PT, 1], f32, name="nwt")
        for _ in range(2):
            # z = y*y*(-0.5*ss); y <- y*(1.5 + z)   (3 DVE ops/step)
            nc.vector.tensor_scalar(nwt[:], rns[:], rns[:], halfss[:],
                                    op0=mult, op1=mult)
            nc.vector.tensor_scalar(nwt[:], nwt[:], 1.5, None, op0=add)
            nc.vector.tensor_mul(rns[:], rns[:], nwt[:])
        ex = persist.tile([PT, K], f32, name="ex")
        nc.vector.tensor_scalar(ex[:], r_ps[:, 0:K], rns[:], 1.0 / 0.05,
                                op0=mult, op1=mult)
        nc.scalar.activation(ex[:], ex[:], AF.Exp)
        ssum = persist.tile([PT, 1], f32, name="ssum")
        nc.vector.tensor_reduce(ssum[:], ex[:], axis=AX.X, op=add)
        rsum = persist.tile([PT, 1], f32, name="rsum")
        nc.vector.reciprocal(rsum[:], ssum[:])
        wmat = persist.tile([PT, K], f32, name="wmat")
        nc.vector.tensor_scalar_mul(wmat[:], ex[:], rsum[:])
        wj = persist.tile([PT, K], f32, name="wj")
        nc.vector.tensor_mul(wj[:], wmat[:], mask_sb[:])
        wcol4 = persist.tile([PT, 1], f32, name="wcol4")
        nc.vector.tensor_reduce(wcol4[:], wj[:], axis=AX.X, op=add)

        # w-scaled x_V^T slabs, bf16 for the output matmul
        xvw4 = persist.tile([PT, QS], bf16, name="xvw4")
        nc.scalar.mul(xvw4[:], stash4[:], wcol4[:])

        # ================= write phase =================
        # s-tile pairs with interleaved chunk matmuls: consecutive MMs
        # target different PE row groups so LDWEIGHTS overlaps in-flight
        # matmuls instead of serializing (lhsT is reloaded per MM).
        with ExitStack() as c2:
            otp = c2.enter_context(
                tc.tile_pool(name="otp", bufs=6, space="PSUM"))
            osb_pool = c2.enter_context(tc.tile_pool(name="osb", bufs=4))

            for ta, tb in ((0, 2), (4, 6), (1, 3), (5, 7)):
                osbs = {ta: osb_pool.tile([PT, D], f16, name="osb"),
                        tb: osb_pool.tile([PT, D], f16, name="osb")}
                for n in range(D // 512):
                    for idx, t in enumerate((ta, tb)):
                        gq, half = t // 2, t % 2
                        o_ps = otp.tile([PT, 512], f32, name="o_ps")
                        nc.tensor.matmul(
                            o_ps[:],
                            xvw4[32 * gq:32 * (gq + 1),
                                 half * PT:(half + 1) * PT],
                            mall_sb[32 * gq:32 * (gq + 1),
                                    n * 512:(n + 1) * 512],
                            start=True, stop=True,
                            tile_position=(32 * gq, 0))
                        dst = osbs[t][:, n * 512:(n + 1) * 512]
                        if (2 * n + idx) % 2 == 0:
                            nc.scalar.copy(dst, o_ps[:])
                        else:
                            nc.vector.tensor_copy(dst, o_ps[:])
                half_d = D // 2
                for t in (ta, tb):
                    nc.sync.dma_start(
                        out[t * PT:(t + 1) * PT, 0:half_d],
                        osbs[t][:, 0:half_d])
                    nc.sync.dma_start(
                        out[t * PT:(t + 1) * PT, half_d:D],
                        osbs[t][:, half_d:D])

    nc.compile()
    return nc


def _get_program():
    if "nc" not in _CACHE:
        _CACHE["nc"] = _build_program()
    return _CACHE["nc"]


def _host_prep(x, U, V, pool, keys, gate_w, gate_b):
    """Parameter folding + per-core shard/layout construction."""
    f32 = np.float32
    f16 = np.float16
    # gate (parameter-only)
    gin = np.concatenate([U.mean(axis=0), V.mean(axis=1)]).astype(f32)
    z = gin @ gate_w[0].astype(f32) + gate_b[0].astype(f32)
    gate = f32(1.0) / (f32(1.0) + np.exp(-z, dtype=f32))
    Ug = (gate * U).astype(f32)

    # mall4 [128, 4096] bf16: 4 replicated slabs of Mall^T [32, 4096],
    # rows 8k+j = (gate*U @ pool[k])[:, j]
    import ml_dtypes
    mall = np.concatenate(
        [(Ug @ pool[k]).T.astype(f32) for k in range(K)], axis=0)
    mall4 = np.ascontiguousarray(np.tile(mall, (4, 1))).astype(
        ml_dtypes.bfloat16)

    # V^T chunk-major, replicated 4x along r: vt[p, c*KR + k*R + r]
    # = V[r, c*128+p]
    vt = np.ascontiguousarray(
        np.tile(V.T.reshape(NCH, PT, R), (1, 1, K))
        .transpose(1, 0, 2).reshape(PT, NCH * KR)).astype(f16)

    # normalized keys, chunk layout [128, K*32]: [p, k*32+c] = kn[k, c*128+p]
    knorm = np.maximum(np.linalg.norm(keys, axis=1, keepdims=True), 1e-8)
    kn = (keys / knorm).astype(f32)
    kcols = np.ascontiguousarray(
        kn.reshape(K, NCH, PT).transpose(2, 0, 1).reshape(PT, K * NCH),
        dtype=f32)

    # mask4 [128, 4]: partition p contributes to expert (p%32)//8
    msk = np.zeros((PT, K), dtype=f32)
    for p in range(PT):
        msk[p, (p % KR) // R] = 1.0

    shared = {"vt": vt, "mall": mall4, "kcols": kcols, "mask": msk}

    # fresh per-call nonce: stale Shared-scratchpad flags from a previous
    # call must never match this call's handshake
    nonce = np.uint32(int.from_bytes(os.urandom(4), "little") | 1)

    in_maps = []
    for core in range(NCORES):
        b, h = divmod(core, 2)
        # x^T fp16, chunk-major: xs[p, c*1024+s] = x[b, h*1024+s, c*128+p]
        xsrd = np.ascontiguousarray(
            x[b, h * SH:(h + 1) * SH, :].T.reshape(NCH, PT, SH)
            .transpose(1, 0, 2).reshape(PT, NCH * SH)).astype(f16)
        if h == 1:
            auxc = np.ascontiguousarray(
                (f32(0.7) * x[b, S - 1, :]).reshape(NCH, PT).T, dtype=f32)
        else:
            auxc = np.zeros((PT, NCH), dtype=f32)
        pr = np.zeros((1, 4), dtype=np.uint32)
        pr[0, 0] = core
        pr[0, 1] = core ^ 1
        pr[0, 2] = nonce
        in_maps.append({"xs": xsrd, "aux": auxc, "peer": pr, **shared})
    return in_maps


def kernel(x, U_shared, V_shared, core_pool, core_keys, gate_w, gate_b):
    global LAST_RESULTS
    from concourse import bass_utils

    x = np.asarray(x, dtype=np.float32)
    U = np.asarray(U_shared, dtype=np.float32)
    V = np.asarray(V_shared, dtype=np.float32)
    pool = np.asarray(core_pool, dtype=np.float32)
    keys = np.asarray(core_keys, dtype=np.float32)
    gw = np.asarray(gate_w, dtype=np.float32)
    gb = np.asarray(gate_b, dtype=np.float32)

    nc = _get_program()
    in_maps = _host_prep(x, U, V, pool, keys, gw, gb)
    res = bass_utils.run_bass_kernel_spmd(
        nc, in_maps, core_ids=list(range(NCORES)))
    LAST_RESULTS = res

    out = np.empty((B, S, D), dtype=np.float32)
    for core in range(NCORES):
        b, h = divmod(core, 2)
        out[b, h * SH:(h + 1) * SH, :] = res.results[core]["out"]
    return out
